# revision 45
# baseline (speedup 1.0000x reference)
"""Block-causal GQA attention on 8 trn2 NeuronCores.

Sharding: core = b*4 + g  (b in {0,1} batch, g in {0..3} kv-head group).
Each core computes, for its batch b and kv group g (4 q-heads, 1 kv head):
    partial_out = softmax_blockcausal(rope(x@Wq_g) @ rope(x@Wk_g)^T) @ (x@Wv_g) @ Wo_g
Host sums the 4 group partials per batch (partials written bf16, summed f32).

Device design (bf16 matmuls, f32 PSUM), single fused schedule keeping PE and
ACT concurrently busy:
  - Host passes x^T, so Q^T/K^T/V^T come out of projections with d on
    partitions; RoPE (sign folded into the sin table) on DVE during PSUM
    eviction.  V^T is DMA-xbar-transposed into V_aug = [V | ones].
  - Weights stream on the scalar-engine DMA queue while x^T streams on the
    sync queue, so the K+V c-outer wave is PE/DMA balanced (~1.3us/chunk).
  - Phase A psum: 6 banks proj ring + 2 banks for early S^T tiles.  The
    attention S^T+exp for head 0 (and head 1) half-0 is woven between Q
    projection j-blocks, so ACT exp work starts ~60us before it otherwise
    would.
  - Phase B: per il-round, [Y|Z][tq,129] += P^T_tile.T @ V_aug for all four
    heads, then O-proj row il immediately (O row needs all heads' Y(il)).
    Normalize via per-partition reciprocal + tensor_scalar, then PE-transpose
    (is_transpose matmul vs identity) + DVE evict into Y^T — no serialized
    DMA transposes on the critical path.  O-proj rows 2..7 are held back as
    PE filler for the exp-bound half-1 windows; row 8+il fires right after
    the last head's Y(1,il).
  - O[t,n] = sum_h Y_h^T.T @ Wo_h accumulated in PSUM over heads; partials
    evicted bf16 and DMA'd from the gpsimd queue.
"""
import os
import sys
import numpy as np

for _p in ("/opt/trn_rl_repo",):
    if _p not in sys.path and os.path.isdir(_p):
        sys.path.insert(0, _p)

import ml_dtypes

BF16 = ml_dtypes.bfloat16

B = 2
T = 2048
C = 2048
HD = 128
NHL = 4           # q heads per core
NT = T // 128     # 16 query/key tiles
NCH = C // 128    # 16 contraction chunks
HW = T // 2       # tq half width
SCALE = 1.0 / float(np.sqrt(np.float32(HD)))

_CACHE = {}


def _build_nc():
    import concourse.bass as bass
    import concourse.mybir as mybir
    import concourse.tile as tile
    from concourse import bacc
    from concourse import masks

    dt = mybir.dt
    f32 = dt.float32
    bf = dt.bfloat16
    Exp = mybir.ActivationFunctionType.Exp

    nc = bacc.Bacc(None, target_bir_lowering=False)

    # weights host-prelaid as [partition, chunk, m] so each DMA is 128 fat
    # contiguous descriptors instead of 2048 small ones
    xT = nc.declare_dram_parameter("xT", [C, T], bf, isOutput=False)
    wq = nc.declare_dram_parameter("wq", [128, NHL, NCH, HD], bf, isOutput=False)
    wk = nc.declare_dram_parameter("wk", [128, NCH, HD], bf, isOutput=False)
    wv = nc.declare_dram_parameter("wv", [128, NCH, HD], bf, isOutput=False)
    wo = nc.declare_dram_parameter("wo", [128, NHL, C], bf, isOutput=False)
    cosT = nc.declare_dram_parameter("cosT", [HD, T], bf, isOutput=False)
    sinT = nc.declare_dram_parameter("sinT", [HD, T], bf, isOutput=False)
    o = nc.declare_dram_parameter("o_part", [T, C], bf, isOutput=True)

    with tile.TileContext(nc) as tc:
        with tc.tile_pool(name="consts", bufs=1) as consts:
            # ---- persistent tiles (survive into phase B) ----
            # V_aug = [V | ones]: col 128 preset to 1, cols 0:128 filled by
            # DMA-transpose from V^T after the V projection.  Rows are 256
            # wide so each tile's dst offset stays 512B-aligned — the DMA
            # xbar transpose corrupts data at unaligned dst offsets.
            vaug_sb = consts.tile([128, NT, 2 * HD], bf, name="vaug_sb")
            nc.vector.memset(vaug_sb[:, :, HD:HD + 1], 1.0)

            ident = consts.tile([128, 128], bf, name="ident")
            masks.make_identity(nc, ident)

            # warm the ACT exp table set before phase A needs it
            dumm = consts.tile([1, 8], f32, name="dumm")
            nc.vector.memset(dumm, 0.0)
            nc.scalar.activation(dumm, dumm, Exp)

            wo_sb = consts.tile([128, NHL, C], bf, name="wo_sb")
            kt_sb = consts.tile([128, T], bf, name="kt_sb")
            qt_sb = [consts.tile([128, T], bf, name=f"qt{h}") for h in range(NHL)]
            yt_sb = [consts.tile([128, T], bf, name=f"yt{h}") for h in range(NHL)]

            # exp'd S^T tiles produced during phase A (heads 0,1 half 0);
            # consumed by phase B's first Y rounds
            p0_sb = [[consts.tile([128, HW], bf, name=f"p0_{h}_{tk}")
                      for tk in range(8)] for h in range(2)]

            # ============ phase A: projections + early attention ==========
            with tc.tile_pool(name="xtp", bufs=1) as xtp, \
                 tc.tile_pool(name="proj", bufs=1) as proj, \
                 tc.tile_pool(name="psA", bufs=1, space="PSUM") as pA:

                # phase-A-only SBUF (freed before phase B)
                wk_sb = proj.tile([128, NCH, HD], bf, name="wk_sb")
                wv_sb = proj.tile([128, NCH, HD], bf, name="wv_sb")
                wq_sb = proj.tile([128, NHL, NCH, HD], bf, name="wq_sb")
                cos_sb = proj.tile([128, T], bf, name="cos_sb")
                sin_sb = proj.tile([128, T], bf, name="sin_sb")
                vt_sb = proj.tile([128, T], bf, name="vt_sb")

                # HBM is the constraint during the wave: only wk/wv (needed
                # by the first matmul) go up front (scalar queue, parallel
                # with the x^T stream on sync).  Everything else issues from
                # gpsimd AFTER chunk 12 lands (dummy-read dependency), so
                # the wave owns the full bandwidth.
                nc.scalar.dma_start(wk_sb, wk[:, :, :])
                nc.scalar.dma_start(wv_sb, wv[:, :, :])

                xt_r = xT.rearrange("(n p) t -> n p t", p=128)
                xt_sb = []
                for cch in range(NCH):
                    xt_c = xtp.tile([128, T], bf, name=f"xt{cch}")
                    nc.sync.dma_start(xt_c, xt_r[cch])
                    xt_sb.append(xt_c)

                # gate the remaining input DMAs behind wave progress.  The
                # scheduler orders by readiness, not emission, so a plain
                # copy "before" the dma_start would be hoisted past it —
                # instead write a probe INTO each DMA's destination tile so
                # the DMA has a real WAW dependency on the gate.
                def gated_dma(dst_probe, dst, src, probe):
                    nc.gpsimd.tensor_copy(dst_probe, probe)
                    nc.gpsimd.dma_start(dst, src)

                probe1 = xt_sb[14][0:1, 0:8]
                gated_dma(cos_sb[0:1, 0:8], cos_sb, cosT[:, :], probe1)
                gated_dma(sin_sb[0:1, 0:8], sin_sb, sinT[:, :], probe1)
                gated_dma(wq_sb[0:1, 0, 0, 0:8], wq_sb[:, 0], wq[:, 0],
                          probe1)

                def rope_evict(ps, jsl, dst):
                    # dst[:, jsl] = ps * cos + rot_half(ps) * sin  (bf16).
                    # ACT does the PSUM eviction; DVE runs at bf16 2x.
                    t0 = proj.tile([128, 512], bf, tag="t0", bufs=4)
                    t1 = proj.tile([128, 512], bf, tag="t1", bufs=4)
                    t2 = proj.tile([128, 512], bf, tag="t2", bufs=4)
                    # sin table halves are pre-swapped on host so each mul
                    # reads both SBUF inputs at the same base partition
                    # (walrus requires equal SBUF base partitions).
                    nc.scalar.copy(t0, ps)
                    nc.vector.tensor_mul(t1, t0, cos_sb[:, jsl])
                    nc.vector.tensor_mul(t2[0:64], t0[64:128], sin_sb[64:128, jsl])
                    nc.vector.tensor_mul(t2[64:128], t0[0:64], sin_sb[0:64, jsl])
                    nc.vector.tensor_add(dst[:, jsl], t1, t2)

                # warm the PE clock (HAM) with throwaway matmuls while the
                # input DMAs stream in; results are never read.  Target is
                # the s0 psum slot (idle until the first woven S^T tile).
                warm_ps = pA.tile([128, HW], f32, tag="s0", bufs=1,
                                  name="warm_ps")

                def warm_mm(n):
                    for _ in range(n):
                        nc.tensor.matmul(warm_ps[0:1, 0:512],
                                         vaug_sb[:, 0, HD:HD + 1],
                                         kt_sb[:, 0:512], start=True, stop=True)

                warm_mm(14)

                # -- wave 1: K (full T) and V (first half) c-outer so PE
                #    starts with the first x^T chunk; 6 psum banks --
                ps_k = [pA.tile([128, 512], f32, tag="pj", bufs=6,
                                name=f"ps_k{j}") for j in range(4)]
                ps_v = [pA.tile([128, 512], f32, tag="pj", bufs=6,
                                name=f"ps_v{j}") for j in range(2)]
                for cch in range(NCH):
                    st, sp = (cch == 0), (cch == NCH - 1)
                    for j in range(4):
                        jsl = slice(512 * j, 512 * (j + 1))
                        nc.tensor.matmul(ps_k[j], wk_sb[:, cch, :],
                                         xt_sb[cch][:, jsl], start=st, stop=sp)
                    for j in range(2):
                        jsl = slice(512 * j, 512 * (j + 1))
                        nc.tensor.matmul(ps_v[j], wv_sb[:, cch, :],
                                         xt_sb[cch][:, jsl], start=st, stop=sp)
                    warm_mm(1)
                for j in range(4):
                    jsl = slice(512 * j, 512 * (j + 1))
                    rope_evict(ps_k[j], jsl, kt_sb)
                for j in range(2):
                    jsl = slice(512 * j, 512 * (j + 1))
                    nc.vector.tensor_copy(vt_sb[:, jsl], ps_v[j])
                # second gate: the remaining weights start only once the K
                # evictions are underway (all x^T chunks long since landed)
                probe2 = kt_sb[0:1, 1536:1544]
                for h in range(1, NHL):
                    gated_dma(wq_sb[0:1, h, 0, 0:8], wq_sb[:, h], wq[:, h],
                              probe2)
                gated_dma(wo_sb[0:1, 0, 0:8], wo_sb, wo[:, :, :], probe2)
                for i in range(8):
                    nc.sync.dma_start_transpose(
                        vaug_sb[:, i, 0:HD], vt_sb[:, 128 * i:128 * (i + 1)]
                    )

                def q_block(h, j, pre=None, mid=None):
                    """One Q psum (c-inner) with optional callables emitted
                    before / midway through the chunk loop (weave points)."""
                    jsl = slice(512 * j, 512 * (j + 1))
                    ps_q = pA.tile([128, 512], f32, tag="pj", bufs=6,
                                   name=f"ps_q{h}_{j}")
                    if pre is not None:
                        pre()
                    for cch in range(NCH):
                        nc.tensor.matmul(
                            ps_q, wq_sb[:, h, cch, :], xt_sb[cch][:, jsl],
                            start=(cch == 0), stop=(cch == NCH - 1))
                        if cch == 7 and mid is not None:
                            mid()
                    rope_evict(ps_q, jsl, qt_sb[h])

                def v2_block(j):
                    # second-half V psums live in the (idle) s0 slot so they
                    # don't wait on the K eviction chain for a pj slot
                    jsl = slice(512 * j, 512 * (j + 1))
                    ps_v2 = pA.tile([128, HW], f32, tag="s0", bufs=1,
                                    name=f"ps_v{j}")
                    for cch in range(NCH):
                        nc.tensor.matmul(ps_v2[:, 0:512], wv_sb[:, cch, :],
                                         xt_sb[cch][:, jsl],
                                         start=(cch == 0), stop=(cch == NCH - 1))
                    nc.vector.tensor_copy(vt_sb[:, jsl], ps_v2[:, 0:512])

                def s_tile_A(h, tk):
                    """Woven S^T + exp for (head h, half 0, key tile tk):
                    p0_sb[h][tk][:, lo:] = exp(K_tk^T.T @ Q^T * scale)."""
                    lo = 128 * tk
                    ps = pA.tile([128, HW], f32, tag="s0", bufs=1, name="ps_s0")
                    chunks = ([(lo, 512), (512, HW)] if lo < 512
                              else [(lo, HW)])
                    for (a, bnd) in chunks:
                        nc.tensor.matmul(
                            ps[:, a:bnd],
                            kt_sb[:, 128 * tk:128 * (tk + 1)],
                            qt_sb[h][:, a:bnd],
                            start=True, stop=True)
                    nc.scalar.activation(p0_sb[h][tk][:, lo:HW], ps[:, lo:HW],
                                         Exp, scale=SCALE)

                # -- V second half + Q head 0 (no attention weave yet) --
                v2_block(2)
                q_block(0, 0)
                v2_block(3)
                q_block(0, 1)
                q_block(0, 2)
                q_block(0, 3)
                for i in range(8, NT):
                    nc.sync.dma_start_transpose(
                        vaug_sb[:, i, 0:HD], vt_sb[:, 128 * i:128 * (i + 1)]
                    )

                # -- Q heads 1..3 with early S^T+exp woven in: head 0's
                #    half-0 tiles during Q(h1)/Q(h2), head 1's during Q(h3) --
                for j in range(4):
                    q_block(1, j, mid=lambda j=j: s_tile_A(0, j))
                for j in range(4):
                    q_block(2, j, mid=lambda j=j: s_tile_A(0, 4 + j))
                for j in range(4):
                    q_block(3, j,
                            pre=lambda j=j: s_tile_A(1, 2 * j),
                            mid=lambda j=j: s_tile_A(1, 2 * j + 1))

                # bridge the A->B psum-pool transition with throwaway
                # matmuls from an already-free pj slot, so PE stays busy
                # (and HAM stays at full rate) while the last projection
                # psums drain and phase B's banks free up
                warm2 = pA.tile([128, 512], f32, tag="pj", bufs=6,
                                name="warm2")
                for _ in range(12):
                    nc.tensor.matmul(warm2[0:1, :],
                                     vaug_sb[:, 0, HD:HD + 1],
                                     kt_sb[:, 0:512], start=True, stop=True)

            # ============ phase B: attention + output projection ==========
            with tc.tile_pool(name="attn", bufs=1) as ap, \
                 tc.tile_pool(name="psB", bufs=1, space="PSUM") as pB:

                # p tiles for S^T produced in phase B (heads 2,3 half 0 and
                # all heads half 1)
                def s_tile_B(h, half, tk, ptile):
                    tq0 = HW * half
                    lo = max(0, 128 * tk - tq0)
                    ps = pB.tile([128, HW], f32, tag="s", bufs=3, name="ps_s")
                    chunks = ([(lo, 512), (512, HW)] if lo < 512
                              else [(lo, HW)])
                    for (a, bnd) in chunks:
                        nc.tensor.matmul(
                            ps[:, a:bnd],
                            kt_sb[:, 128 * tk:128 * (tk + 1)],
                            qt_sb[h][:, tq0 + a:tq0 + bnd],
                            start=True, stop=True)
                    nc.scalar.activation(ptile[:, lo:HW], ps[:, lo:HW],
                                         Exp, scale=SCALE)

                pend_tp = []

                def y_group(h, half, il, tiles, tp="defer"):
                    """One [Y|Z] accumulation + normalize + transpose-out.
                    tp="dma": xbar-DMA transpose on the sync queue — only
                    for groups whose O-proj consumer is far enough away to
                    ride out the serialized ~1.2us/transpose queue.
                    tp="defer": PE-transpose (ident matmul) + DVE evict,
                    deferred via pend_tp so it never waits on the DVE
                    normalize of its own group."""
                    gi = 8 * half + il
                    ps_yz = pB.tile([128, 512], f32, tag="b1", bufs=2,
                                    name="ps_yz")
                    for tk in range(gi + 1):
                        nc.tensor.matmul(
                            ps_yz[:, 0:HD + 1],
                            tiles[tk][:, 128 * il:128 * (il + 1)],
                            vaug_sb[:, tk, 0:HD + 1],
                            start=(tk == 0), stop=(tk == gi))
                    rz = ap.tile([128, 1], f32, tag="rz", bufs=8)
                    nc.vector.reciprocal(rz, ps_yz[:, HD:HD + 1])
                    ysb = ap.tile([128, HD], bf, tag="ysb", bufs=8)
                    nc.vector.tensor_scalar_mul(ysb, ps_yz[:, 0:HD], rz)
                    if tp == "dma":
                        nc.sync.dma_start_transpose(
                            yt_sb[h][:, 128 * gi:128 * (gi + 1)], ysb)
                    else:
                        pend_tp.append((h, gi, ysb))

                def flush_tp():
                    if not pend_tp:
                        return
                    h, gi, ysb = pend_tp.pop(0)
                    tp = pB.tile([128, 512], bf, tag="b1", bufs=2,
                                 name="tp")
                    nc.tensor.transpose(tp[:, 0:128], ysb, ident)
                    nc.vector.tensor_copy(
                        yt_sb[h][:, 128 * gi:128 * (gi + 1)], tp[:, 0:128])

                def och(ti, n, evict="dve"):
                    """One O-proj psum chunk (quarter row tile) — the
                    granular PE filler unit (~0.85us).  Evictions stay off
                    ACT in exp-bound blocks; the pure-PE tail alternates
                    DVE/ACT."""
                    tsl = slice(128 * ti, 128 * (ti + 1))
                    nsl = slice(512 * n, 512 * (n + 1))
                    ps_o = pB.tile([128, 512], f32, tag="b1",
                                   bufs=2, name="ps_o")
                    for h in range(NHL):
                        nc.tensor.matmul(
                            ps_o, yt_sb[h][:, tsl], wo_sb[:, h, nsl],
                            start=(h == 0), stop=(h == NHL - 1))
                    ob = ap.tile([128, 512], bf, tag="ob", bufs=12)
                    if evict == "dve" or n % 2 == 0:
                        nc.vector.tensor_copy(ob, ps_o)
                    else:
                        nc.scalar.copy(ob, ps_o)
                    nc.gpsimd.dma_start(o[tsl, nsl], ob)

                def oproj(ti, dve_only=False):
                    for n in range(C // 512):
                        och(ti, n, evict="mix")

                # p tiles: phase-A tiles for heads 0,1 half 0; fresh ring
                # tiles for everything else
                pt = {}
                pt[(0, 0)] = p0_sb[0]
                pt[(1, 0)] = p0_sb[1]

                def make_ptiles(h, half):
                    tiles = [ap.tile([128, HW], bf, tag="p", bufs=34,
                                     name=f"p_{h}_{half}_{tk}")
                             for tk in range(8 + 8 * half)]
                    pt[(h, half)] = tiles
                    return tiles

                # -- attention blocks, each S^T+exp window packed with PE
                #    filler sized tk-by-tk to the exp widths: Y groups are
                #    woven LARGEST-FIRST against the wide early exps, and
                #    O-proj rows are spread as quarter-row chunks.  Heads
                #    0/1 half-0 have no S^T here (exp'd in phase A) — their
                #    Y groups are the filler for the first two windows. --

                # half-0 Y groups: large il woven first (against the wide
                # early exps).  il<=2 PE-transpose (their O rows 0-2 come
                # too soon for the serialized DMA-transpose queue); il>=3
                # ride the sync-queue xbar DMA.
                def h0_mode(il):
                    return "dma" if il >= 3 else "defer"

                # block (2,0): S^T(2,0) + y(0,0), y(1,0) filler (reversed)
                t20 = make_ptiles(2, 0)
                for tk in range(8):
                    s_tile_B(2, 0, tk, t20[tk])
                    y_group(0, 0, 7 - tk, pt[(0, 0)], h0_mode(7 - tk))
                    y_group(1, 0, 7 - tk, pt[(1, 0)], h0_mode(7 - tk))
                    flush_tp()

                # block (3,0): S^T(3,0) + y(2,0) (reversed), y(3,0) (lag 2)
                t30 = make_ptiles(3, 0)
                for tk in range(8):
                    s_tile_B(3, 0, tk, t30[tk])
                    y_group(2, 0, 7 - tk, pt[(2, 0)], h0_mode(7 - tk))
                    if tk >= 2:
                        y_group(3, 0, tk - 2, pt[(3, 0)], h0_mode(tk - 2))
                    flush_tp()

                # block (0,1): S^T(0,1) + O rows 0-2 as chunks + y(3,0) tail
                t01 = make_ptiles(0, 1)
                for tk in range(NT):
                    s_tile_B(0, 1, tk, t01[tk])
                    flush_tp()
                    if tk == 1:
                        y_group(3, 0, 6, pt[(3, 0)], "dma")
                    if tk == 2:
                        y_group(3, 0, 7, pt[(3, 0)], "dma")
                    if 3 <= tk <= 14:
                        och((tk - 3) // 4, (tk - 3) % 4)

                # blocks (h,1) for h=1..3: S^T + y(h-1,1) + O-row chunks.
                # y(h-1,1) groups ride DMA transposes — their O rows (8-15)
                # are all in the tail, several blocks away.  Only head 3's
                # half-1 groups (tail-coupled) PE-transpose via pend_tp.
                for h in range(1, NHL):
                    tiles = make_ptiles(h, 1)
                    nch = [(3 + 2 * (h - 1) + n // 4, n % 4)
                           for n in range(8 if h < 3 else 4)]
                    for tk in range(NT):
                        s_tile_B(h, 1, tk, tiles[tk])
                        if tk % 2 == 1 and tk <= 11:
                            y_group(h - 1, 1, tk // 2, pt[(h - 1, 1)], "dma")
                        if (tk == 15 or (tk % 2 == 0 and tk >= 2)) and nch:
                            ti, n = nch.pop(0)
                            och(ti, n)
                        if h < 3:
                            if tk == 13:
                                y_group(h - 1, 1, 6, pt[(h - 1, 1)], "dma")
                            if tk == 15:
                                y_group(h - 1, 1, 7, pt[(h - 1, 1)], "dma")
                        else:
                            if tk == 12:
                                y_group(2, 1, 6, pt[(2, 1)], "dma")
                            if tk == 13:
                                y_group(3, 1, 0, pt[(3, 1)])
                            if tk == 14:
                                y_group(2, 1, 7, pt[(2, 1)], "dma")
                            if tk == 15:
                                y_group(3, 1, 1, pt[(3, 1)])

                # -- tail: last head's remaining Y, transposes flushed one
                #    step behind, O-proj rows two Y-groups behind (pure PE,
                #    ACT idle) --
                flush_tp()
                for il in range(2, 8):
                    y_group(3, 1, il, pt[(3, 1)])
                    flush_tp()
                    oproj(il + 6, dve_only=False)
                flush_tp()
                oproj(14, dve_only=False)
                oproj(15, dve_only=False)

    nc.finalize()
    return nc


def _tables():
    freqs = 1.0 / (10000.0 ** (np.arange(0, HD, 2, dtype=np.float32) / HD))
    t = np.arange(T, dtype=np.float32)
    emb = np.outer(t, freqs)                  # [T, 64]
    cos_t = np.cos(emb).T.astype(np.float32)  # [64, T]
    sin_t = np.sin(emb).T.astype(np.float32)
    cosT = np.ascontiguousarray(np.concatenate([cos_t, cos_t], 0)).astype(BF16)
    # halves swapped: row d holds the factor multiplying t0[(d+64)%128]
    # when writing t2[d ^ 64 half]; see rope_evict
    sinT = np.ascontiguousarray(np.concatenate([sin_t, -sin_t], 0)).astype(BF16)
    return cosT, sinT


def _get_nc():
    if "nc" not in _CACHE:
        _CACHE["nc"] = _build_nc()
    return _CACHE["nc"]


def kernel(x, Wq, Wk, Wv, Wo, _trace=False):
    from concourse.bass_utils import run_bass_kernel_spmd

    x = np.asarray(x, dtype=np.float32)
    cosT, sinT = _tables()

    def chunked(w):
        # [K, m] -> [128, K//128, m] (partition-major, contiguous)
        k, m = w.shape
        return np.ascontiguousarray(
            w.reshape(k // 128, 128, m).transpose(1, 0, 2)).astype(BF16)

    in_maps = []
    for core in range(8):
        b, g = divmod(core, 4)
        wq_g = Wq[:, 512 * g:512 * (g + 1)]
        in_maps.append({
            "xT": np.ascontiguousarray(x[b].T).astype(BF16),
            "wq": np.ascontiguousarray(np.stack(
                [chunked(wq_g[:, 128 * h:128 * (h + 1)]) for h in range(NHL)],
                axis=1)),
            "wk": chunked(Wk[:, 128 * g:128 * (g + 1)]),
            "wv": chunked(Wv[:, 128 * g:128 * (g + 1)]),
            "wo": chunked(Wo[512 * g:512 * (g + 1), :]),
            "cosT": cosT,
            "sinT": sinT,
        })

    nc = _get_nc()
    res = run_bass_kernel_spmd(nc, in_maps, list(range(8)), trace=_trace)
    parts = [res.results[c]["o_part"].astype(np.float32) for c in range(8)]
    out = np.empty((B, T, C), dtype=np.float32)
    for b in range(B):
        out[b] = parts[4 * b] + parts[4 * b + 1] + parts[4 * b + 2] + parts[4 * b + 3]
    if _trace:
        return out, res
    return out


# revision 46
# speedup vs baseline: 1.1696x; 1.1696x over previous
"""Block-causal GQA attention on 8 trn2 NeuronCores.

Sharding: core = b*4 + g  (b in {0,1} batch, g in {0..3} kv-head group).
Each core computes, for its batch b and kv group g (4 q-heads, 1 kv head):
    partial_out = softmax_blockcausal(rope(x@Wq_g) @ rope(x@Wk_g)^T) @ (x@Wv_g) @ Wo_g
Host sums the 4 group partials per batch (partials written bf16, summed f32).

Device design (bf16 matmuls, f32 PSUM), single fused schedule keeping PE and
ACT concurrently busy:
  - Host passes x^T, so Q^T/K^T/V^T come out of projections with d on
    partitions; RoPE (sign folded into the sin table) on DVE during PSUM
    eviction.  V^T is DMA-xbar-transposed into V_aug = [V | ones].
  - Weights stream on the scalar-engine DMA queue while x^T streams on the
    sync queue, so the K+V c-outer wave is PE/DMA balanced (~1.3us/chunk).
  - Phase A psum: 6 banks proj ring + 2 banks for early S^T tiles.  The
    attention S^T+exp for head 0 (and head 1) half-0 is woven between Q
    projection j-blocks, so ACT exp work starts ~60us before it otherwise
    would.
  - Phase B: per il-round, [Y|Z][tq,129] += P^T_tile.T @ V_aug for all four
    heads, then O-proj row il immediately (O row needs all heads' Y(il)).
    Normalize via per-partition reciprocal + tensor_scalar, then PE-transpose
    (is_transpose matmul vs identity) + DVE evict into Y^T — no serialized
    DMA transposes on the critical path.  O-proj rows 2..7 are held back as
    PE filler for the exp-bound half-1 windows; row 8+il fires right after
    the last head's Y(1,il).
  - O[t,n] = sum_h Y_h^T.T @ Wo_h accumulated in PSUM over heads; partials
    evicted bf16 and DMA'd from the gpsimd queue.
"""
import os
import sys
import numpy as np

for _p in ("/opt/trn_rl_repo",):
    if _p not in sys.path and os.path.isdir(_p):
        sys.path.insert(0, _p)

import ml_dtypes

BF16 = ml_dtypes.bfloat16

B = 2
T = 2048
C = 2048
HD = 128
NHL = 4           # q heads per core
NT = T // 128     # 16 query/key tiles
NCH = C // 128    # 16 contraction chunks
HW = T // 2       # tq half width
SCALE = 1.0 / float(np.sqrt(np.float32(HD)))

_CACHE = {}


def _build_nc():
    import concourse.bass as bass
    import concourse.mybir as mybir
    import concourse.tile as tile
    from concourse import bacc
    from concourse import masks

    dt = mybir.dt
    f32 = dt.float32
    bf = dt.bfloat16
    Exp = mybir.ActivationFunctionType.Exp

    nc = bacc.Bacc(None, target_bir_lowering=False)

    # weights host-prelaid as [partition, chunk, m] so each DMA is 128 fat
    # contiguous descriptors instead of 2048 small ones
    xT = nc.declare_dram_parameter("xT", [C, T], bf, isOutput=False)
    wq = nc.declare_dram_parameter("wq", [128, NHL, NCH, HD], bf, isOutput=False)
    wk = nc.declare_dram_parameter("wk", [128, NCH, HD], bf, isOutput=False)
    wv = nc.declare_dram_parameter("wv", [128, NCH, HD], bf, isOutput=False)
    wo = nc.declare_dram_parameter("wo", [128, NHL, C], bf, isOutput=False)
    cosT = nc.declare_dram_parameter("cosT", [HD, T], bf, isOutput=False)
    sinT = nc.declare_dram_parameter("sinT", [HD, T], bf, isOutput=False)
    o = nc.declare_dram_parameter("o_part", [T, C], bf, isOutput=True)

    with tile.TileContext(nc) as tc:
        with tc.tile_pool(name="consts", bufs=1) as consts:
            # ---- persistent tiles (survive into phase B) ----
            # V_aug = [V | ones]: col 128 preset to 1, cols 0:128 filled by
            # DMA-transpose from V^T after the V projection.  Rows are 256
            # wide so each tile's dst offset stays 512B-aligned — the DMA
            # xbar transpose corrupts data at unaligned dst offsets.
            vaug_sb = consts.tile([128, NT, 2 * HD], bf, name="vaug_sb")
            nc.vector.memset(vaug_sb[:, :, HD:HD + 1], 1.0)

            ident = consts.tile([128, 128], bf, name="ident")
            masks.make_identity(nc, ident)

            # warm the ACT exp table set before phase A needs it
            dumm = consts.tile([1, 8], f32, name="dumm")
            nc.vector.memset(dumm, 0.0)
            nc.scalar.activation(dumm, dumm, Exp)

            wo_sb = consts.tile([128, NHL, C], bf, name="wo_sb")
            kt_sb = consts.tile([128, T], bf, name="kt_sb")
            qt_sb = [consts.tile([128, T], bf, name=f"qt{h}") for h in range(NHL)]
            yt_sb = [consts.tile([128, T], bf, name=f"yt{h}") for h in range(NHL)]

            # exp'd S^T tiles produced during phase A (heads 0,1 half 0);
            # consumed by phase B's first Y rounds
            p0_sb = [[consts.tile([128, HW], bf, name=f"p0_{h}_{tk}")
                      for tk in range(8)] for h in range(2)]

            # ============ phase A: projections + early attention ==========
            with tc.tile_pool(name="xtp", bufs=1) as xtp, \
                 tc.tile_pool(name="proj", bufs=1) as proj, \
                 tc.tile_pool(name="psA", bufs=1, space="PSUM") as pA:

                # phase-A-only SBUF (freed before phase B)
                wk_sb = proj.tile([128, NCH, HD], bf, name="wk_sb")
                wv_sb = proj.tile([128, NCH, HD], bf, name="wv_sb")
                wq_sb = proj.tile([128, NHL, NCH, HD], bf, name="wq_sb")
                cos_sb = proj.tile([128, T], bf, name="cos_sb")
                sin_sb = proj.tile([128, T], bf, name="sin_sb")
                vt_sb = proj.tile([128, T], bf, name="vt_sb")

                # HBM is the constraint during the wave: only wk/wv (needed
                # by the first matmul) go up front (scalar queue, parallel
                # with the x^T stream on sync).  Everything else issues from
                # gpsimd AFTER chunk 12 lands (dummy-read dependency), so
                # the wave owns the full bandwidth.
                nc.scalar.dma_start(wk_sb, wk[:, :, :])
                nc.scalar.dma_start(wv_sb, wv[:, :, :])

                xt_r = xT.rearrange("(n p) t -> n p t", p=128)
                xt_sb = []
                for cch in range(NCH):
                    xt_c = xtp.tile([128, T], bf, name=f"xt{cch}")
                    nc.sync.dma_start(xt_c, xt_r[cch])
                    xt_sb.append(xt_c)

                # gate the remaining input DMAs behind wave progress.  The
                # scheduler orders by readiness, not emission, so a plain
                # copy "before" the dma_start would be hoisted past it —
                # instead write a probe INTO each DMA's destination tile so
                # the DMA has a real WAW dependency on the gate.
                def gated_dma(dst_probe, dst, src, probe):
                    nc.gpsimd.tensor_copy(dst_probe, probe)
                    nc.gpsimd.dma_start(dst, src)

                probe1 = xt_sb[14][0:1, 0:8]
                gated_dma(cos_sb[0:1, 0:8], cos_sb, cosT[:, :], probe1)
                gated_dma(sin_sb[0:1, 0:8], sin_sb, sinT[:, :], probe1)
                gated_dma(wq_sb[0:1, 0, 0, 0:8], wq_sb[:, 0], wq[:, 0],
                          probe1)

                def rope_evict(ps, jsl, dst):
                    # dst[:, jsl] = ps * cos + rot_half(ps) * sin  (bf16).
                    # ACT does the PSUM eviction; DVE runs at bf16 2x.
                    t0 = proj.tile([128, 512], bf, tag="t0", bufs=4)
                    t1 = proj.tile([128, 512], bf, tag="t1", bufs=4)
                    t2 = proj.tile([128, 512], bf, tag="t2", bufs=4)
                    # sin table halves are pre-swapped on host so each mul
                    # reads both SBUF inputs at the same base partition
                    # (walrus requires equal SBUF base partitions).
                    nc.scalar.copy(t0, ps)
                    nc.vector.tensor_mul(t1, t0, cos_sb[:, jsl])
                    nc.vector.tensor_mul(t2[0:64], t0[64:128], sin_sb[64:128, jsl])
                    nc.vector.tensor_mul(t2[64:128], t0[0:64], sin_sb[0:64, jsl])
                    nc.vector.tensor_add(dst[:, jsl], t1, t2)

                # warm the PE clock (HAM) with throwaway matmuls while the
                # input DMAs stream in; results are never read.  Target is
                # the s0 psum slot (idle until the first woven S^T tile).
                warm_ps = pA.tile([128, HW], f32, tag="s0", bufs=1,
                                  name="warm_ps")

                def warm_mm(n):
                    for _ in range(n):
                        nc.tensor.matmul(warm_ps[0:1, 0:512],
                                         vaug_sb[:, 0, HD:HD + 1],
                                         kt_sb[:, 0:512], start=True, stop=True)

                warm_mm(14)

                # -- wave 1: K (full T) and V (first half) c-outer so PE
                #    starts with the first x^T chunk; 6 psum banks --
                ps_k = [pA.tile([128, 512], f32, tag="pj", bufs=6,
                                name=f"ps_k{j}") for j in range(4)]
                ps_v = [pA.tile([128, 512], f32, tag="pj", bufs=6,
                                name=f"ps_v{j}") for j in range(2)]
                for cch in range(NCH):
                    st, sp = (cch == 0), (cch == NCH - 1)
                    for j in range(4):
                        jsl = slice(512 * j, 512 * (j + 1))
                        nc.tensor.matmul(ps_k[j], wk_sb[:, cch, :],
                                         xt_sb[cch][:, jsl], start=st, stop=sp)
                    for j in range(2):
                        jsl = slice(512 * j, 512 * (j + 1))
                        nc.tensor.matmul(ps_v[j], wv_sb[:, cch, :],
                                         xt_sb[cch][:, jsl], start=st, stop=sp)
                    warm_mm(1)
                for j in range(4):
                    jsl = slice(512 * j, 512 * (j + 1))
                    rope_evict(ps_k[j], jsl, kt_sb)
                for j in range(2):
                    jsl = slice(512 * j, 512 * (j + 1))
                    nc.vector.tensor_copy(vt_sb[:, jsl], ps_v[j])
                # second gate: the remaining weights start only once the K
                # evictions are underway (all x^T chunks long since landed)
                probe2 = kt_sb[0:1, 1536:1544]
                for h in range(1, NHL):
                    gated_dma(wq_sb[0:1, h, 0, 0:8], wq_sb[:, h], wq[:, h],
                              probe2)
                gated_dma(wo_sb[0:1, 0, 0:8], wo_sb, wo[:, :, :], probe2)
                for i in range(8):
                    nc.sync.dma_start_transpose(
                        vaug_sb[:, i, 0:HD], vt_sb[:, 128 * i:128 * (i + 1)]
                    )

                def q_block(h, j, pre=None, mid=None):
                    """One Q psum (c-inner) with optional callables emitted
                    before / midway through the chunk loop (weave points)."""
                    jsl = slice(512 * j, 512 * (j + 1))
                    ps_q = pA.tile([128, 512], f32, tag="pj", bufs=6,
                                   name=f"ps_q{h}_{j}")
                    if pre is not None:
                        pre()
                    for cch in range(NCH):
                        nc.tensor.matmul(
                            ps_q, wq_sb[:, h, cch, :], xt_sb[cch][:, jsl],
                            start=(cch == 0), stop=(cch == NCH - 1))
                        if cch == 7 and mid is not None:
                            mid()
                    rope_evict(ps_q, jsl, qt_sb[h])

                def v2_block(j):
                    # second-half V psums live in the (idle) s0 slot so they
                    # don't wait on the K eviction chain for a pj slot
                    jsl = slice(512 * j, 512 * (j + 1))
                    ps_v2 = pA.tile([128, HW], f32, tag="s0", bufs=1,
                                    name=f"ps_v{j}")
                    for cch in range(NCH):
                        nc.tensor.matmul(ps_v2[:, 0:512], wv_sb[:, cch, :],
                                         xt_sb[cch][:, jsl],
                                         start=(cch == 0), stop=(cch == NCH - 1))
                    nc.vector.tensor_copy(vt_sb[:, jsl], ps_v2[:, 0:512])

                def s_tile_A(h, tk):
                    """Woven S^T + exp for (head h, half 0, key tile tk):
                    p0_sb[h][tk][:, lo:] = exp(K_tk^T.T @ Q^T * scale)."""
                    lo = 128 * tk
                    ps = pA.tile([128, HW], f32, tag="s0", bufs=1, name="ps_s0")
                    chunks = ([(lo, 512), (512, HW)] if lo < 512
                              else [(lo, HW)])
                    for (a, bnd) in chunks:
                        nc.tensor.matmul(
                            ps[:, a:bnd],
                            kt_sb[:, 128 * tk:128 * (tk + 1)],
                            qt_sb[h][:, a:bnd],
                            start=True, stop=True)
                    nc.scalar.activation(p0_sb[h][tk][:, lo:HW], ps[:, lo:HW],
                                         Exp, scale=SCALE)

                # -- V second half + Q head 0 (no attention weave yet) --
                v2_block(2)
                q_block(0, 0)
                v2_block(3)
                q_block(0, 1)
                q_block(0, 2)
                q_block(0, 3)
                for i in range(8, NT):
                    nc.sync.dma_start_transpose(
                        vaug_sb[:, i, 0:HD], vt_sb[:, 128 * i:128 * (i + 1)]
                    )

                # -- Q heads 1..3 with early S^T+exp woven in: head 0's
                #    half-0 tiles during Q(h1)/Q(h2), head 1's during Q(h3) --
                for j in range(4):
                    q_block(1, j, mid=lambda j=j: s_tile_A(0, j))
                for j in range(4):
                    q_block(2, j, mid=lambda j=j: s_tile_A(0, 4 + j))
                for j in range(4):
                    q_block(3, j,
                            pre=lambda j=j: s_tile_A(1, 2 * j),
                            mid=lambda j=j: s_tile_A(1, 2 * j + 1))

                # bridge the A->B psum-pool transition with throwaway
                # matmuls from an already-free pj slot, so PE stays busy
                # (and HAM stays at full rate) while the last projection
                # psums drain and phase B's banks free up
                warm2 = pA.tile([128, 512], f32, tag="pj", bufs=6,
                                name="warm2")
                for _ in range(12):
                    nc.tensor.matmul(warm2[0:1, :],
                                     vaug_sb[:, 0, HD:HD + 1],
                                     kt_sb[:, 0:512], start=True, stop=True)

            # ============ phase B: attention + output projection ==========
            with tc.tile_pool(name="attn", bufs=1) as ap, \
                 tc.tile_pool(name="psB", bufs=1, space="PSUM") as pB:

                # p tiles for S^T produced in phase B (heads 2,3 half 0 and
                # all heads half 1)
                def s_tile_B(h, half, tk, ptile):
                    tq0 = HW * half
                    lo = max(0, 128 * tk - tq0)
                    ps = pB.tile([128, HW], f32, tag="s", bufs=3, name="ps_s")
                    chunks = ([(lo, 512), (512, HW)] if lo < 512
                              else [(lo, HW)])
                    for (a, bnd) in chunks:
                        nc.tensor.matmul(
                            ps[:, a:bnd],
                            kt_sb[:, 128 * tk:128 * (tk + 1)],
                            qt_sb[h][:, tq0 + a:tq0 + bnd],
                            start=True, stop=True)
                    nc.scalar.activation(ptile[:, lo:HW], ps[:, lo:HW],
                                         Exp, scale=SCALE)

                pend_tp = []

                def y_group(h, half, il, tiles, tp="defer"):
                    """One [Y|Z] accumulation + normalize + transpose-out.
                    tp="dma": xbar-DMA transpose on the sync queue — only
                    for groups whose O-proj consumer is far enough away to
                    ride out the serialized ~1.2us/transpose queue.
                    tp="defer": PE-transpose (ident matmul) + DVE evict,
                    deferred via pend_tp so it never waits on the DVE
                    normalize of its own group."""
                    gi = 8 * half + il
                    ps_yz = pB.tile([128, 512], f32, tag="b1", bufs=2,
                                    name="ps_yz")
                    for tk in range(gi + 1):
                        nc.tensor.matmul(
                            ps_yz[:, 0:HD + 1],
                            tiles[tk][:, 128 * il:128 * (il + 1)],
                            vaug_sb[:, tk, 0:HD + 1],
                            start=(tk == 0), stop=(tk == gi))
                    rz = ap.tile([128, 1], f32, tag="rz", bufs=8)
                    nc.vector.reciprocal(rz, ps_yz[:, HD:HD + 1])
                    ysb = ap.tile([128, HD], bf, tag="ysb", bufs=8)
                    nc.vector.tensor_scalar_mul(ysb, ps_yz[:, 0:HD], rz)
                    if tp == "dma":
                        nc.sync.dma_start_transpose(
                            yt_sb[h][:, 128 * gi:128 * (gi + 1)], ysb)
                    else:
                        pend_tp.append((h, gi, ysb))

                def flush_tp():
                    if not pend_tp:
                        return
                    h, gi, ysb = pend_tp.pop(0)
                    tp = pB.tile([128, 512], bf, tag="b1", bufs=2,
                                 name="tp")
                    nc.tensor.transpose(tp[:, 0:128], ysb, ident)
                    nc.vector.tensor_copy(
                        yt_sb[h][:, 128 * gi:128 * (gi + 1)], tp[:, 0:128])

                def och(ti, n, evict="dve"):
                    """One O-proj psum chunk (quarter row tile) — the
                    granular PE filler unit (~0.85us).  Evictions stay off
                    ACT in exp-bound blocks; the pure-PE tail alternates
                    DVE/ACT."""
                    tsl = slice(128 * ti, 128 * (ti + 1))
                    nsl = slice(512 * n, 512 * (n + 1))
                    ps_o = pB.tile([128, 512], f32, tag="b1",
                                   bufs=2, name="ps_o")
                    for h in range(NHL):
                        nc.tensor.matmul(
                            ps_o, yt_sb[h][:, tsl], wo_sb[:, h, nsl],
                            start=(h == 0), stop=(h == NHL - 1))
                    ob = ap.tile([128, 512], bf, tag="ob", bufs=12)
                    if evict == "dve" or n % 2 == 0:
                        nc.vector.tensor_copy(ob, ps_o)
                    else:
                        nc.scalar.copy(ob, ps_o)
                    nc.gpsimd.dma_start(o[tsl, nsl], ob)

                def oproj(ti, dve_only=False):
                    for n in range(C // 512):
                        och(ti, n, evict="mix")

                # p tiles: phase-A tiles for heads 0,1 half 0; fresh ring
                # tiles for everything else
                pt = {}
                pt[(0, 0)] = p0_sb[0]
                pt[(1, 0)] = p0_sb[1]

                def make_ptiles(h, half):
                    tiles = [ap.tile([128, HW], bf, tag="p", bufs=34,
                                     name=f"p_{h}_{half}_{tk}")
                             for tk in range(8 + 8 * half)]
                    pt[(h, half)] = tiles
                    return tiles

                # -- attention blocks, each S^T+exp window packed with PE
                #    filler sized tk-by-tk to the exp widths: Y groups are
                #    woven LARGEST-FIRST against the wide early exps, and
                #    O-proj rows are spread as quarter-row chunks.  Heads
                #    0/1 half-0 have no S^T here (exp'd in phase A) — their
                #    Y groups are the filler for the first two windows. --

                # half-0 Y groups: large il woven first (against the wide
                # early exps).  il<=2 PE-transpose (their O rows 0-2 come
                # too soon for the serialized DMA-transpose queue); il>=3
                # ride the sync-queue xbar DMA.
                def h0_mode(il):
                    return "dma" if il >= 3 else "defer"

                # block (2,0): S^T(2,0) + y(0,0), y(1,0) filler (reversed)
                t20 = make_ptiles(2, 0)
                for tk in range(8):
                    s_tile_B(2, 0, tk, t20[tk])
                    y_group(0, 0, 7 - tk, pt[(0, 0)], h0_mode(7 - tk))
                    y_group(1, 0, 7 - tk, pt[(1, 0)], h0_mode(7 - tk))
                    flush_tp()

                # block (3,0): S^T(3,0) + y(2,0) (reversed), y(3,0) (lag 2)
                t30 = make_ptiles(3, 0)
                for tk in range(8):
                    s_tile_B(3, 0, tk, t30[tk])
                    y_group(2, 0, 7 - tk, pt[(2, 0)], h0_mode(7 - tk))
                    if tk >= 2:
                        y_group(3, 0, tk - 2, pt[(3, 0)], h0_mode(tk - 2))
                    flush_tp()

                # block (0,1): S^T(0,1) + O rows 0-2 as chunks + y(3,0) tail
                t01 = make_ptiles(0, 1)
                for tk in range(NT):
                    s_tile_B(0, 1, tk, t01[tk])
                    flush_tp()
                    if tk == 1:
                        y_group(3, 0, 6, pt[(3, 0)], "dma")
                    if tk == 2:
                        y_group(3, 0, 7, pt[(3, 0)], "dma")
                    if 3 <= tk <= 14:
                        och((tk - 3) // 4, (tk - 3) % 4)

                # blocks (h,1) for h=1..3: S^T + y(h-1,1) + O-row chunks.
                # The PE transposes (flush_tp) double as PE filler for
                # these ACT-bound windows — removing them tips the blocks
                # into stall/HAM-throttle spirals (measured +44us).
                for h in range(1, NHL):
                    tiles = make_ptiles(h, 1)
                    nch = [(3 + 2 * (h - 1) + n // 4, n % 4)
                           for n in range(8 if h < 3 else 4)]
                    for tk in range(NT):
                        s_tile_B(h, 1, tk, tiles[tk])
                        if tk % 2 == 1 and tk <= 11:
                            flush_tp()
                            y_group(h - 1, 1, tk // 2, pt[(h - 1, 1)])
                        if (tk == 15 or (tk % 2 == 0 and tk >= 2)) and nch:
                            ti, n = nch.pop(0)
                            och(ti, n)
                        if h < 3:
                            if tk == 13:
                                flush_tp()
                                y_group(h - 1, 1, 6, pt[(h - 1, 1)])
                            if tk == 15:
                                flush_tp()
                                y_group(h - 1, 1, 7, pt[(h - 1, 1)])
                        else:
                            if tk == 12:
                                flush_tp()
                                y_group(2, 1, 6, pt[(2, 1)])
                            if tk == 13:
                                flush_tp()
                                y_group(3, 1, 0, pt[(3, 1)])
                            if tk == 14:
                                flush_tp()
                                y_group(2, 1, 7, pt[(2, 1)])
                            if tk == 15:
                                flush_tp()
                                y_group(3, 1, 1, pt[(3, 1)])

                # -- tail: last head's remaining Y two O-rows ahead of the
                #    O-proj rows that consume them (pure PE, ACT idle) --
                for il in range(2, 8):
                    flush_tp()
                    y_group(3, 1, il, pt[(3, 1)])
                    oproj(il + 6, dve_only=False)
                flush_tp()
                oproj(14, dve_only=False)
                oproj(15, dve_only=False)

    nc.finalize()
    return nc


def _tables():
    freqs = 1.0 / (10000.0 ** (np.arange(0, HD, 2, dtype=np.float32) / HD))
    t = np.arange(T, dtype=np.float32)
    emb = np.outer(t, freqs)                  # [T, 64]
    cos_t = np.cos(emb).T.astype(np.float32)  # [64, T]
    sin_t = np.sin(emb).T.astype(np.float32)
    cosT = np.ascontiguousarray(np.concatenate([cos_t, cos_t], 0)).astype(BF16)
    # halves swapped: row d holds the factor multiplying t0[(d+64)%128]
    # when writing t2[d ^ 64 half]; see rope_evict
    sinT = np.ascontiguousarray(np.concatenate([sin_t, -sin_t], 0)).astype(BF16)
    return cosT, sinT


def _get_nc():
    if "nc" not in _CACHE:
        _CACHE["nc"] = _build_nc()
    return _CACHE["nc"]


def kernel(x, Wq, Wk, Wv, Wo, _trace=False):
    from concourse.bass_utils import run_bass_kernel_spmd

    x = np.asarray(x, dtype=np.float32)
    cosT, sinT = _tables()

    def chunked(w):
        # [K, m] -> [128, K//128, m] (partition-major, contiguous)
        k, m = w.shape
        return np.ascontiguousarray(
            w.reshape(k // 128, 128, m).transpose(1, 0, 2)).astype(BF16)

    in_maps = []
    for core in range(8):
        b, g = divmod(core, 4)
        wq_g = Wq[:, 512 * g:512 * (g + 1)]
        in_maps.append({
            "xT": np.ascontiguousarray(x[b].T).astype(BF16),
            "wq": np.ascontiguousarray(np.stack(
                [chunked(wq_g[:, 128 * h:128 * (h + 1)]) for h in range(NHL)],
                axis=1)),
            "wk": chunked(Wk[:, 128 * g:128 * (g + 1)]),
            "wv": chunked(Wv[:, 128 * g:128 * (g + 1)]),
            "wo": chunked(Wo[512 * g:512 * (g + 1), :]),
            "cosT": cosT,
            "sinT": sinT,
        })

    nc = _get_nc()
    res = run_bass_kernel_spmd(nc, in_maps, list(range(8)), trace=_trace)
    parts = [res.results[c]["o_part"].astype(np.float32) for c in range(8)]
    out = np.empty((B, T, C), dtype=np.float32)
    for b in range(B):
        out[b] = parts[4 * b] + parts[4 * b + 1] + parts[4 * b + 2] + parts[4 * b + 3]
    if _trace:
        return out, res
    return out
